# revision 2
# baseline (speedup 1.0000x reference)
"""AutoCorrelation Trainium2 kernel (Bass/Tile, 8 NeuronCores).

Math (per row r of [B*L, 512] with D=512):
  corr_r = irfft(rfft(q_r) * conj(rfft(k_r)))            (circular cross-correlation)
  mean_r = mean(top7(corr_r))
  w0 = sigmoid(corr - mean); out = v*w0 + roll(v,-1,L)*(1-w0)
     = v + sigmoid(mean - corr) * (roll(v) - v)

Implementation:
  - DFT/iDFT as fp16 matmuls on the PE with a packed-real 512-point basis:
    packed[f] layout: A-block f=0..255 = Re[f] (A[0]=Re0), B-block = Im[f]
    (B[0]=Re256).  Forward: QF^T[fpacked, row] = W^T q^T via
    lhsT=W-block, rhs=qT (DMA-xbar-transposed q16).  Product spectrum
    P = QF o conj(KF) elementwise on DVE (block formulas + 2 f=0 fixups).
    Inverse: corr[row, t] via lhsT=P-chunk, rhs=C-block -> PSUM fp32,
    already in row-major layout.
  - top-7 mean via the DVE max8 instruction reading corr in PSUM.
  - sigmoid on ACT directly off PSUM with per-partition bias = +mean/scale=-1.
  - Row interleave: partition p = row//64, subblock s = row%64 makes
    roll(v,-1) = "read subblock s+1" (same partitions); batch wraps and the
    s=63 edge are handled by one small shifted DMA load (vsh).
  - Sharding: batch-parallel, 4 batches per core, no communication.
"""
import numpy as np

B, L, D = 32, 2048, 512
N_CORES = 8
BPC = B // N_CORES            # batches per core
ROWS = BPC * L                # 8192 rows per core
NSUB = 64                     # subblocks (s = row % 64)
P = 128                       # partitions (p = row // 64)
SB_GROUP = 8                  # subblocks per DMA superblock
NSUPER = NSUB // SB_GROUP     # 8 superblocks
TOPK = 7

_CACHE = {}


def _dft_consts():
    """Packed-real DFT matrices W [512 feat, 512 packed] and C [512 packed, 512 t]."""
    j = np.arange(D)[:, None].astype(np.float64)
    f = np.arange(256)[None, :].astype(np.float64)
    Wc = np.cos(-2 * np.pi * j * f / D)
    Ws = np.sin(-2 * np.pi * j * f / D)
    WB = Ws.copy()
    WB[:, 0] = np.cos(np.pi * j[:, 0])          # B0 row: Re256
    W = np.concatenate([Wc, WB], axis=1)        # [512, 512]
    t = np.arange(D)[None, :].astype(np.float64)
    fc = np.arange(256)[:, None].astype(np.float64)
    Ca = np.cos(2 * np.pi * fc * t / D) * 2 / D
    Ca[0] = 1.0 / D
    Cb = -np.sin(2 * np.pi * fc * t / D) * 2 / D
    Cb[0] = np.cos(np.pi * t[0]) / D
    C = np.concatenate([Ca, Cb], axis=0)        # [512, 512]
    return W.astype(np.float32), C.astype(np.float32)


def _build_nc():
    import concourse.bacc as bacc
    import concourse.mybir as mybir
    from concourse.tile import TileContext

    f16 = mybir.dt.float16
    f32 = mybir.dt.float32

    W, C = _dft_consts()
    # W16[p, jj, fp]  = W[jj*128+p, fp]   (lhsT blocks for GEMM-1)
    W16 = W.reshape(4, P, D).transpose(1, 0, 2).astype(np.float16).copy()
    # C16[p, ff, t]   = C[ff*128+p, t]    (rhs blocks for GEMM-2)
    C16 = C.reshape(4, P, D).transpose(1, 0, 2).astype(np.float16).copy()

    nc = bacc.Bacc()
    q_d = nc.dram_tensor("query", [ROWS, D], f32, kind="ExternalInput")
    k_d = nc.dram_tensor("key", [ROWS, D], f32, kind="ExternalInput")
    v_d = nc.dram_tensor("value", [ROWS, D], f32, kind="ExternalInput")
    o_d = nc.dram_tensor("out", [ROWS, D], f32, kind="ExternalOutput")
    w_t = nc.inline_tensor(W16, name="Wdft")
    c_t = nc.inline_tensor(C16, name="Cdft")

    # interleaved views: [p, s, c] with row = 64*p + s
    qv = q_d.rearrange("(p s) c -> p s c", s=NSUB)
    kv = k_d.rearrange("(p s) c -> p s c", s=NSUB)
    vv = v_d.rearrange("(p s) c -> p s c", s=NSUB)
    ov = o_d.rearrange("(p s) c -> p s c", s=NSUB)

    with TileContext(nc) as tc:
        with (
            tc.tile_pool(name="consts", bufs=1) as consts,
            tc.tile_pool(name="io", bufs=2) as io,
            tc.tile_pool(name="work", bufs=3) as work,
            tc.tile_pool(name="small", bufs=8) as small,
            tc.tile_pool(name="ps", bufs=1, space="PSUM") as psp,
            tc.tile_pool(name="psc", bufs=4, space="PSUM") as pscp,
        ):
            wt = consts.tile([P, 4, D], f16)      # W16
            ct = consts.tile([P, 4, D], f16)      # C16
            nc.sync.dma_start(out=wt, in_=w_t[:, :, :])
            nc.sync.dma_start(out=ct, in_=c_t[:, :, :])

            # vsh[p] = v[row 64p+64] ; fix wraps at p in {31,63,95,127} <- batch starts
            vsh = consts.tile([P, D], f16)
            vflat = v_d  # [ROWS, D]
            nc.gpsimd.dma_start(
                out=vsh[0:127], in_=vflat.rearrange("(a b) c -> a b c", b=NSUB)[1:128, 0]
            )  # rows 64,128,...,8128
            nc.gpsimd.dma_start(
                out=vsh.rearrange("(w u) c -> w u c", u=32)[:, 31:32, :].rearrange("w u c -> (w u) c"),
                in_=vflat.rearrange("(b t) c -> b t c", t=L)[:, 0:1, :].rearrange("b t c -> (b t) c"),
            )  # vsh[31,63,95,127] <- v rows {0, 2048, 4096, 6144}

            def load_super(sbi):
                sl = slice(sbi * SB_GROUP, (sbi + 1) * SB_GROUP)
                q16 = io.tile([P, SB_GROUP, D], f16, tag="q16")
                k16 = io.tile([P, SB_GROUP, D], f16, tag="k16")
                v16 = io.tile([P, SB_GROUP, D], f16, tag="v16")
                nc.gpsimd.dma_start(out=q16, in_=qv[:, sl, :])
                nc.gpsimd.dma_start(out=k16, in_=kv[:, sl, :])
                nc.gpsimd.dma_start(out=v16, in_=vv[:, sl, :])
                return q16, k16, v16

            def compute_group(q16, k16, gl):
                """gl: local group index (0..3) inside superblock; returns w1 tiles
                for the two subblocks (local indices 2gl, 2gl+1)."""
                qT = work.tile([P, 4, 256], f16, tag="qT")
                kT = work.tile([P, 4, 256], f16, tag="kT")
                for sp in range(2):
                    nc.sync.dma_start_transpose(
                        qT[:, :, sp * P:(sp + 1) * P], q16[:, 2 * gl + sp, :])
                    nc.sync.dma_start_transpose(
                        kT[:, :, sp * P:(sp + 1) * P], k16[:, 2 * gl + sp, :])

                psq = psp.tile([P, 4, 256], f32, tag="psq")
                psk = psp.tile([P, 4, 256], f32, tag="psk")
                for mm in range(4):
                    for jj in range(4):
                        nc.tensor.matmul(psq[:, mm, :], wt[:, jj, mm * P:(mm + 1) * P],
                                         qT[:, jj, :], start=(jj == 0), stop=(jj == 3))
                for mm in range(4):
                    for jj in range(4):
                        nc.tensor.matmul(psk[:, mm, :], wt[:, jj, mm * P:(mm + 1) * P],
                                         kT[:, jj, :], start=(jj == 0), stop=(jj == 3))

                qf = work.tile([P, 4, 256], f16, tag="qf")
                kf = work.tile([P, 4, 256], f16, tag="kf")
                nc.scalar.copy(qf, psq)
                nc.scalar.copy(kf, psk)

                # products: Pa = QA.KA + QB.KB ; Pb = QB.KA - QA.KB
                t1 = work.tile([P, 2, 256], f16, tag="t1")
                t2 = work.tile([P, 2, 256], f16, tag="t2")
                pt = work.tile([P, 4, 256], f16, tag="pt")
                QA, QB = qf[:, 0:2, :], qf[:, 2:4, :]
                KA, KB = kf[:, 0:2, :], kf[:, 2:4, :]
                nc.vector.tensor_mul(t1, QA, KA)
                nc.vector.tensor_mul(t2, QB, KB)
                nc.vector.tensor_add(pt[:, 0:2, :], t1, t2)
                nc.vector.tensor_mul(t1, QB, KA)
                nc.vector.tensor_mul(t2, QA, KB)
                nc.vector.tensor_sub(pt[:, 2:4, :], t1, t2)
                # f=0 fixups (partition 0 of slices 0 and 2)
                nc.vector.tensor_mul(pt[0:1, 0, :], qf[0:1, 0, :], kf[0:1, 0, :])
                nc.vector.tensor_mul(pt[0:1, 2, :], qf[0:1, 2, :], kf[0:1, 2, :])

                w1s = []
                for sp in range(2):
                    cps = pscp.tile([P, D], f32, tag="cps")
                    for ff in range(4):
                        nc.tensor.matmul(cps, pt[:, ff, sp * P:(sp + 1) * P],
                                         ct[:, ff, :], start=(ff == 0), stop=(ff == 3))
                    mx = small.tile([P, 8], f32, tag="mx")
                    nc.vector.max(out=mx, in_=cps)
                    sm = small.tile([P, 1], f32, tag="sm")
                    nc.vector.reduce_sum(sm, mx[:, 0:TOPK], axis=mybir.AxisListType.X)
                    pm = small.tile([P, 1], f32, tag="pm")
                    nc.vector.tensor_scalar_mul(pm, sm, 1.0 / TOPK)
                    w1 = work.tile([P, D], f16, tag="w1")
                    nc.scalar.activation(w1, cps, mybir.ActivationFunctionType.Sigmoid,
                                         bias=pm, scale=-1.0)
                    w1s.append(w1)
                return w1s

            def combine(v16, sl, w1, vnext, o16):
                """out[:, sl] = v16[:, sl] + w1*(vnext - v16[:, sl])"""
                dt_ = work.tile([P, D], f16, tag="dt")
                zt = work.tile([P, D], f16, tag="zt")
                nc.vector.tensor_sub(dt_, vnext, v16[:, sl, :])
                nc.vector.tensor_mul(zt, w1, dt_)
                nc.gpsimd.tensor_add(o16[:, sl, :], v16[:, sl, :], zt)

            prev = None  # (q16,k16,v16,o16, w1 list, sbi)
            for sbi in range(NSUPER):
                q16, k16, v16 = load_super(sbi)
                o16 = io.tile([P, SB_GROUP, D], f16, tag="o16")
                w1l = []
                for gl in range(4):
                    w1l += compute_group(q16, k16, gl)
                if prev is not None:
                    pq, pk, pv, po, pw, psbi = prev
                    for sl in range(SB_GROUP):
                        vnext = pv[:, sl + 1, :] if sl < SB_GROUP - 1 else v16[:, 0, :]
                        combine(pv, sl, pw[sl], vnext, po)
                    nc.gpsimd.dma_start(
                        out=ov[:, psbi * SB_GROUP:(psbi + 1) * SB_GROUP, :], in_=po)
                prev = (q16, k16, v16, o16, w1l, sbi)

            pq, pk, pv, po, pw, psbi = prev
            for sl in range(SB_GROUP):
                vnext = pv[:, sl + 1, :] if sl < SB_GROUP - 1 else vsh
                combine(pv, sl, pw[sl], vnext, po)
            nc.gpsimd.dma_start(
                out=ov[:, psbi * SB_GROUP:(psbi + 1) * SB_GROUP, :], in_=po)

    nc.finalize()
    return nc


def kernel(query, key, value):
    import sys
    if "/opt/trn_rl_repo" not in sys.path:
        sys.path.insert(0, "/opt/trn_rl_repo")
    from concourse.bass_utils import run_bass_kernel_spmd

    if "nc" not in _CACHE:
        _CACHE["nc"] = _build_nc()
    nc = _CACHE["nc"]

    q = np.ascontiguousarray(np.asarray(query, dtype=np.float32).reshape(B, L, D))
    k = np.ascontiguousarray(np.asarray(key, dtype=np.float32).reshape(B, L, D))
    v = np.ascontiguousarray(np.asarray(value, dtype=np.float32).reshape(B, L, D))

    in_maps = []
    for c in range(N_CORES):
        sl = slice(c * BPC, (c + 1) * BPC)
        in_maps.append({
            "query": q[sl].reshape(ROWS, D),
            "key": k[sl].reshape(ROWS, D),
            "value": v[sl].reshape(ROWS, D),
        })
    res = run_bass_kernel_spmd(nc, in_maps, core_ids=list(range(N_CORES)),
                               trace=bool(_CACHE.get("trace")))
    _CACHE["last_result"] = res
    out = np.empty((B, L, D), dtype=np.float32)
    for c in range(N_CORES):
        out[c * BPC:(c + 1) * BPC] = res.results[c]["out"].reshape(BPC, L, D)
    return out


# revision 4
# speedup vs baseline: 163.5274x; 163.5274x over previous
"""AutoCorrelation Trainium2 kernel (Bass/Tile, 8 NeuronCores).

Math (per row r of [B*L, 512] with D=512):
  corr_r = irfft(rfft(q_r) * conj(rfft(k_r)))            (circular cross-correlation)
  mean_r = mean(top7(corr_r))
  w0 = sigmoid(corr - mean); out = v*w0 + roll(v,-1,L)*(1-w0)
     = v + sigmoid(mean - corr) * (roll(v) - v)

Implementation:
  - DFT/iDFT as fp16 matmuls on the PE with a packed-real 512-point basis:
    packed[f] layout: A-block f=0..255 = Re[f] (A[0]=Re0), B-block = Im[f]
    (B[0]=Re256).  Forward: QF^T[fpacked, row] = W^T q^T via
    lhsT=W-block, rhs=qT (DMA-xbar-transposed q16).  Product spectrum
    P = QF o conj(KF) elementwise on DVE (block formulas + 2 f=0 fixups).
    Inverse: corr[row, t] via lhsT=P-chunk, rhs=C-block -> PSUM fp32,
    already in row-major layout.
  - top-7 mean via the DVE max8 instruction reading corr in PSUM.
  - sigmoid on ACT directly off PSUM with per-partition bias = +mean/scale=-1.
  - Row interleave: partition p = row//64, subblock s = row%64 makes
    roll(v,-1) = "read subblock s+1" (same partitions); batch wraps and the
    s=63 edge are handled by one small shifted DMA load (vsh).
  - Sharding: batch-parallel, 4 batches per core, no communication.
"""
import numpy as np

B, L, D = 32, 2048, 512
N_CORES = 8
BPC = B // N_CORES            # batches per core
ROWS = BPC * L                # 8192 rows per core
NSUB = 64                     # subblocks (s = row % 64)
P = 128                       # partitions (p = row // 64)
SB_GROUP = 8                  # subblocks per DMA superblock
NSUPER = NSUB // SB_GROUP     # 8 superblocks
TOPK = 7

_CACHE = {}


def _dft_consts():
    """Packed-real DFT matrices W [512 feat, 512 packed] and C [512 packed, 512 t]."""
    j = np.arange(D)[:, None].astype(np.float64)
    f = np.arange(256)[None, :].astype(np.float64)
    Wc = np.cos(-2 * np.pi * j * f / D)
    Ws = np.sin(-2 * np.pi * j * f / D)
    WB = Ws.copy()
    WB[:, 0] = np.cos(np.pi * j[:, 0])          # B0 row: Re256
    W = np.concatenate([Wc, WB], axis=1)        # [512, 512]
    t = np.arange(D)[None, :].astype(np.float64)
    fc = np.arange(256)[:, None].astype(np.float64)
    Ca = np.cos(2 * np.pi * fc * t / D) * 2 / D
    Ca[0] = 1.0 / D
    Cb = -np.sin(2 * np.pi * fc * t / D) * 2 / D
    Cb[0] = np.cos(np.pi * t[0]) / D
    C = np.concatenate([Ca, Cb], axis=0)        # [512, 512]
    return W.astype(np.float32), C.astype(np.float32)


def _build_nc(n_iter=1):
    import concourse.bacc as bacc
    import concourse.mybir as mybir
    from concourse.tile import TileContext

    f16 = mybir.dt.float16
    f32 = mybir.dt.float32

    W, C = _dft_consts()
    # W16[p, jj, fp]  = W[jj*128+p, fp]   (lhsT blocks for GEMM-1)
    W16 = W.reshape(4, P, D).transpose(1, 0, 2).astype(np.float16).copy()
    # C16[p, ff, t]   = C[ff*128+p, t]    (rhs blocks for GEMM-2)
    C16 = C.reshape(4, P, D).transpose(1, 0, 2).astype(np.float16).copy()

    nc = bacc.Bacc()
    q_d = nc.dram_tensor("query", [ROWS, D], f32, kind="ExternalInput")
    k_d = nc.dram_tensor("key", [ROWS, D], f32, kind="ExternalInput")
    v_d = nc.dram_tensor("value", [ROWS, D], f32, kind="ExternalInput")
    o_d = nc.dram_tensor("out", [ROWS, D], f32, kind="ExternalOutput")
    w_t = nc.inline_tensor(W16, name="Wdft")
    c_t = nc.inline_tensor(C16, name="Cdft")

    # interleaved views: [p, s, c] with row = 64*p + s
    qv = q_d.rearrange("(p s) c -> p s c", s=NSUB)
    kv = k_d.rearrange("(p s) c -> p s c", s=NSUB)
    vv = v_d.rearrange("(p s) c -> p s c", s=NSUB)
    ov = o_d.rearrange("(p s) c -> p s c", s=NSUB)

    with TileContext(nc) as tc:
        with (
            tc.tile_pool(name="consts", bufs=1) as consts,
            tc.tile_pool(name="io", bufs=2) as io,
            tc.tile_pool(name="work", bufs=3) as work,
            tc.tile_pool(name="small", bufs=8) as small,
            tc.tile_pool(name="ps", bufs=1, space="PSUM") as psp,
            tc.tile_pool(name="psc", bufs=4, space="PSUM") as pscp,
        ):
            wt = consts.tile([P, 4, D], f16)      # W16
            ct = consts.tile([P, 4, D], f16)      # C16
            nc.sync.dma_start(out=wt, in_=w_t[:, :, :])
            nc.sync.dma_start(out=ct, in_=c_t[:, :, :])

            # vsh[p] = v[row 64p+64] ; fix wraps at p in {31,63,95,127} <- batch starts
            vsh = consts.tile([P, D], f16)
            vflat = v_d  # [ROWS, D]
            nc.gpsimd.dma_start(
                out=vsh[0:127], in_=vflat.rearrange("(a b) c -> a b c", b=NSUB)[1:128, 0]
            )  # rows 64,128,...,8128
            nc.gpsimd.dma_start(
                out=vsh.rearrange("(w u) c -> w u c", u=32)[:, 31:32, :].rearrange("w u c -> (w u) c"),
                in_=vflat.rearrange("(b t) c -> b t c", t=L)[:, 0:1, :].rearrange("b t c -> (b t) c"),
            )  # vsh[31,63,95,127] <- v rows {0, 2048, 4096, 6144}

            def load_super(sbi):
                sl = slice(sbi * SB_GROUP, (sbi + 1) * SB_GROUP)
                q16 = io.tile([P, SB_GROUP, D], f16, tag="q16")
                k16 = io.tile([P, SB_GROUP, D], f16, tag="k16")
                v16 = io.tile([P, SB_GROUP, D], f16, tag="v16")
                nc.gpsimd.dma_start(out=q16, in_=qv[:, sl, :])
                nc.gpsimd.dma_start(out=k16, in_=kv[:, sl, :])
                nc.gpsimd.dma_start(out=v16, in_=vv[:, sl, :])
                return q16, k16, v16

            def compute_group(q16, k16, gl):
                """gl: local group index (0..3) inside superblock; returns w1 tiles
                for the two subblocks (local indices 2gl, 2gl+1)."""
                qT = work.tile([P, 4, 256], f16, tag="qT")
                kT = work.tile([P, 4, 256], f16, tag="kT")
                for sp in range(2):
                    nc.sync.dma_start_transpose(
                        qT[:, :, sp * P:(sp + 1) * P], q16[:, 2 * gl + sp, :])
                    nc.sync.dma_start_transpose(
                        kT[:, :, sp * P:(sp + 1) * P], k16[:, 2 * gl + sp, :])

                psq = psp.tile([P, 4, 256], f32, tag="psq")
                psk = psp.tile([P, 4, 256], f32, tag="psk")
                for mm in range(4):
                    for jj in range(4):
                        nc.tensor.matmul(psq[:, mm, :], wt[:, jj, mm * P:(mm + 1) * P],
                                         qT[:, jj, :], start=(jj == 0), stop=(jj == 3))
                for mm in range(4):
                    for jj in range(4):
                        nc.tensor.matmul(psk[:, mm, :], wt[:, jj, mm * P:(mm + 1) * P],
                                         kT[:, jj, :], start=(jj == 0), stop=(jj == 3))

                qf = work.tile([P, 4, 256], f16, tag="qf")
                kf = work.tile([P, 4, 256], f16, tag="kf")
                nc.scalar.copy(qf, psq)
                nc.scalar.copy(kf, psk)

                # products: Pa = QA.KA + QB.KB ; Pb = QB.KA - QA.KB
                t1 = work.tile([P, 2, 256], f16, tag="t1")
                t2 = work.tile([P, 2, 256], f16, tag="t2")
                pt = work.tile([P, 4, 256], f16, tag="pt")
                QA, QB = qf[:, 0:2, :], qf[:, 2:4, :]
                KA, KB = kf[:, 0:2, :], kf[:, 2:4, :]
                nc.vector.tensor_mul(t1, QA, KA)
                nc.vector.tensor_mul(t2, QB, KB)
                nc.vector.tensor_add(pt[:, 0:2, :], t1, t2)
                nc.vector.tensor_mul(t1, QB, KA)
                nc.vector.tensor_mul(t2, QA, KB)
                nc.vector.tensor_sub(pt[:, 2:4, :], t1, t2)
                # f=0 fixups (partition 0 of slices 0 and 2)
                nc.vector.tensor_mul(pt[0:1, 0, :], qf[0:1, 0, :], kf[0:1, 0, :])
                nc.vector.tensor_mul(pt[0:1, 2, :], qf[0:1, 2, :], kf[0:1, 2, :])

                w1s = []
                for sp in range(2):
                    cps = pscp.tile([P, D], f32, tag="cps")
                    for ff in range(4):
                        nc.tensor.matmul(cps, pt[:, ff, sp * P:(sp + 1) * P],
                                         ct[:, ff, :], start=(ff == 0), stop=(ff == 3))
                    mx = small.tile([P, 8], f32, tag="mx")
                    nc.vector.max(out=mx, in_=cps)
                    sm = small.tile([P, 1], f32, tag="sm")
                    nc.vector.reduce_sum(sm, mx[:, 0:TOPK], axis=mybir.AxisListType.X)
                    pm = small.tile([P, 1], f32, tag="pm")
                    nc.vector.tensor_scalar_mul(pm, sm, 1.0 / TOPK)
                    w1 = work.tile([P, D], f16, tag="w1")
                    nc.scalar.activation(w1, cps, mybir.ActivationFunctionType.Sigmoid,
                                         bias=pm, scale=-1.0)
                    w1s.append(w1)
                return w1s

            def combine(v16, sl, w1, vnext, o16):
                """out[:, sl] = v16[:, sl] + w1*(vnext - v16[:, sl])"""
                dt_ = work.tile([P, D], f16, tag="dt")
                zt = work.tile([P, D], f16, tag="zt")
                nc.vector.tensor_sub(dt_, vnext, v16[:, sl, :])
                nc.vector.tensor_mul(zt, w1, dt_)
                nc.gpsimd.tensor_add(o16[:, sl, :], v16[:, sl, :], zt)

            def pipeline():
                prev = None  # (q16,k16,v16,o16, w1 list, sbi)
                for sbi in range(NSUPER):
                    q16, k16, v16 = load_super(sbi)
                    o16 = io.tile([P, SB_GROUP, D], f16, tag="o16")
                    w1l = []
                    for gl in range(4):
                        w1l += compute_group(q16, k16, gl)
                    if prev is not None:
                        pq, pk, pv, po, pw, psbi = prev
                        for sl in range(SB_GROUP):
                            vnext = pv[:, sl + 1, :] if sl < SB_GROUP - 1 else v16[:, 0, :]
                            combine(pv, sl, pw[sl], vnext, po)
                        nc.gpsimd.dma_start(
                            out=ov[:, psbi * SB_GROUP:(psbi + 1) * SB_GROUP, :], in_=po)
                    prev = (q16, k16, v16, o16, w1l, sbi)

                pq, pk, pv, po, pw, psbi = prev
                for sl in range(SB_GROUP):
                    vnext = pv[:, sl + 1, :] if sl < SB_GROUP - 1 else vsh
                    combine(pv, sl, pw[sl], vnext, po)
                nc.gpsimd.dma_start(
                    out=ov[:, psbi * SB_GROUP:(psbi + 1) * SB_GROUP, :], in_=po)

            if n_iter == 1:
                pipeline()
            else:
                with tc.For_i(0, n_iter, 1):
                    pipeline()

    nc.finalize()
    return nc


def kernel(query, key, value):
    import sys
    if "/opt/trn_rl_repo" not in sys.path:
        sys.path.insert(0, "/opt/trn_rl_repo")
    from concourse.bass_utils import run_bass_kernel_spmd

    if "nc" not in _CACHE:
        _CACHE["nc"] = _build_nc()
    nc = _CACHE["nc"]

    q = np.ascontiguousarray(np.asarray(query, dtype=np.float32).reshape(B, L, D))
    k = np.ascontiguousarray(np.asarray(key, dtype=np.float32).reshape(B, L, D))
    v = np.ascontiguousarray(np.asarray(value, dtype=np.float32).reshape(B, L, D))

    in_maps = []
    for c in range(N_CORES):
        sl = slice(c * BPC, (c + 1) * BPC)
        in_maps.append({
            "query": q[sl].reshape(ROWS, D),
            "key": k[sl].reshape(ROWS, D),
            "value": v[sl].reshape(ROWS, D),
        })
    res = run_bass_kernel_spmd(nc, in_maps, core_ids=list(range(N_CORES)),
                               trace=bool(_CACHE.get("trace")))
    _CACHE["last_result"] = res
    out = np.empty((B, L, D), dtype=np.float32)
    for c in range(N_CORES):
        out[c * BPC:(c + 1) * BPC] = res.results[c]["out"].reshape(BPC, L, D)
    return out


# revision 16
# speedup vs baseline: 192.4854x; 1.1771x over previous
"""AutoCorrelation Trainium2 kernel (Bass/Tile, 8 NeuronCores).

Math (per row r of [B*L, 512] with D=512):
  corr_r = irfft(rfft(q_r) * conj(rfft(k_r)))            (circular cross-correlation)
  mean_r = mean(top7(corr_r))
  w0 = sigmoid(corr - mean); out = v*w0 + roll(v,-1,L)*(1-w0)
     = v + sigmoid(mean - corr) * (roll(v) - v)

Implementation:
  - DFT/iDFT as fp16 matmuls on the PE with a packed-real 512-point basis:
    packed[f] layout: A-block f=0..255 = Re[f] (A[0]=Re0), B-block = Im[f]
    (B[0]=Re256).  Forward: QF^T[fpacked, row] = W^T q^T via
    lhsT=W-block, rhs=qT (DMA-xbar-transposed q16).  Product spectrum
    P = QF o conj(KF) elementwise on DVE (block formulas + 2 f=0 fixups).
    Inverse: corr[row, t] via lhsT=P-chunk, rhs=C-block -> PSUM fp32,
    already in row-major layout.
  - top-7 mean via the DVE max8 instruction reading corr in PSUM.
  - sigmoid on ACT directly off PSUM with per-partition bias = +mean/scale=-1.
  - Row interleave: partition p = row//64, subblock s = row%64 makes
    roll(v,-1) = "read subblock s+1" (same partitions); batch wraps and the
    s=63 edge are handled by one small shifted DMA load (vsh).
  - Sharding: batch-parallel, 4 batches per core, no communication.
"""
import numpy as np

B, L, D = 32, 2048, 512
N_CORES = 8
BPC = B // N_CORES            # batches per core
ROWS = BPC * L                # 8192 rows per core
NSUB = 64                     # subblocks (s = row % 64)
P = 128                       # partitions (p = row // 64)
SB_GROUP = 8                  # subblocks per DMA superblock
NSUPER = NSUB // SB_GROUP     # 8 superblocks
TOPK = 7

_CACHE = {}


def _dft_consts():
    """Packed-real DFT matrices W [512 feat, 512 packed] and C [512 packed, 512 t]."""
    j = np.arange(D)[:, None].astype(np.float64)
    f = np.arange(256)[None, :].astype(np.float64)
    Wc = np.cos(-2 * np.pi * j * f / D)
    Ws = np.sin(-2 * np.pi * j * f / D)
    WB = Ws.copy()
    WB[:, 0] = np.cos(np.pi * j[:, 0])          # B0 row: Re256
    W = np.concatenate([Wc, WB], axis=1)        # [512, 512]
    t = np.arange(D)[None, :].astype(np.float64)
    fc = np.arange(256)[:, None].astype(np.float64)
    Ca = np.cos(2 * np.pi * fc * t / D) * 2 / D
    Ca[0] = 1.0 / D
    Cb = -np.sin(2 * np.pi * fc * t / D) * 2 / D
    Cb[0] = np.cos(np.pi * t[0]) / D
    C = np.concatenate([Ca, Cb], axis=0)        # [512, 512]
    return W.astype(np.float32), C.astype(np.float32)


def _build_nc(n_iter=1):
    import os
    import concourse.bacc as bacc
    import concourse.mybir as mybir
    from concourse.tile import TileContext

    ABL = set(os.environ.get("ABL", "").split(","))

    f16 = mybir.dt.float16
    f32 = mybir.dt.float32

    W, C = _dft_consts()
    # W16[p, jj, fp]  = W[jj*128+p, fp]   (lhsT blocks for GEMM-1)
    W16 = W.reshape(4, P, D).transpose(1, 0, 2).astype(np.float16).copy()
    # C16[p, ff, t]   = C[ff*128+p, t]    (rhs blocks for GEMM-2)
    C16 = C.reshape(4, P, D).transpose(1, 0, 2).astype(np.float16).copy()

    nc = bacc.Bacc()
    q_d = nc.dram_tensor("query", [ROWS, D], f32, kind="ExternalInput")
    k_d = nc.dram_tensor("key", [ROWS, D], f32, kind="ExternalInput")
    v_d = nc.dram_tensor("value", [ROWS, D], f32, kind="ExternalInput")
    o_d = nc.dram_tensor("out", [ROWS, D], f32, kind="ExternalOutput")
    w_t = nc.inline_tensor(W16, name="Wdft")
    c_t = nc.inline_tensor(C16, name="Cdft")

    # interleaved views: [p, s, c] with row = 64*p + s
    qv = q_d.rearrange("(p s) c -> p s c", s=NSUB)
    kv = k_d.rearrange("(p s) c -> p s c", s=NSUB)
    vv = v_d.rearrange("(p s) c -> p s c", s=NSUB)
    ov = o_d.rearrange("(p s) c -> p s c", s=NSUB)

    with TileContext(nc) as tc:
        with (
            tc.tile_pool(name="consts", bufs=1) as consts,
            tc.tile_pool(name="io", bufs=2) as io,
            tc.tile_pool(name="work", bufs=3) as work,
            tc.tile_pool(name="small", bufs=8) as small,
            tc.tile_pool(name="ps", bufs=4, space="PSUM") as psp,
        ):
            wt = consts.tile([P, 4, D], f16)      # W16
            ct = consts.tile([P, 4, D], f16)      # C16
            nc.sync.dma_start(out=wt, in_=w_t[:, :, :])
            nc.sync.dma_start(out=ct, in_=c_t[:, :, :])

            # vsh[p] = v[row 64p+64] ; fix wraps at p in {31,63,95,127} <- batch starts
            vsh = consts.tile([P, D], f16)
            vflat = v_d  # [ROWS, D]
            nc.gpsimd.dma_start(
                out=vsh[0:127], in_=vflat.rearrange("(a b) c -> a b c", b=NSUB)[1:128, 0]
            )  # rows 64,128,...,8128
            nc.gpsimd.dma_start(
                out=vsh.rearrange("(w u) c -> w u c", u=32)[:, 31:32, :].rearrange("w u c -> (w u) c"),
                in_=vflat.rearrange("(b t) c -> b t c", t=L)[:, 0:1, :].rearrange("b t c -> (b t) c"),
            )  # vsh[31,63,95,127] <- v rows {0, 2048, 4096, 6144}

            def load_super(sbi):
                sl = slice(sbi * SB_GROUP, (sbi + 1) * SB_GROUP)
                q16 = io.tile([P, SB_GROUP, D], f16, tag="q16")
                k16 = io.tile([P, SB_GROUP, D], f16, tag="k16")
                v16 = io.tile([P, SB_GROUP, D], f16, tag="v16")
                if "loadhalf" in ABL:
                    nc.gpsimd.dma_start(out=q16, in_=qv[:, sl, :])
                    return q16, q16, q16
                nc.gpsimd.dma_start(out=q16, in_=qv[:, sl, :])
                nc.gpsimd.dma_start(out=k16, in_=kv[:, sl, :])
                nc.gpsimd.dma_start(out=v16, in_=vv[:, sl, :])
                return q16, k16, v16

            def compute_group(q16, k16, gl, w1sb):
                """gl: local group index (0..3) inside superblock; writes w1 into
                w1sb slices for the two subblocks (local indices 2gl, 2gl+1)."""
                qT = work.tile([P, 4, 256], f16, tag="qT")
                kT = work.tile([P, 4, 256], f16, tag="kT")
                for sp in range(2):
                    nc.sync.dma_start_transpose(
                        qT[:, :, sp * P:(sp + 1) * P], q16[:, 2 * gl + sp, :])
                    nc.sync.dma_start_transpose(
                        kT[:, :, sp * P:(sp + 1) * P], k16[:, 2 * gl + sp, :])

                psq = psp.tile([P, 4, 256], f32, tag="ps2bank")
                psk = psp.tile([P, 4, 256], f32, tag="ps2bank")
                for mm in range(4):
                    for jj in range(4):
                        nc.tensor.matmul(psq[:, mm, :], wt[:, jj, mm * P:(mm + 1) * P],
                                         qT[:, jj, :], start=(jj == 0), stop=(jj == 3))
                for mm in range(4):
                    for jj in range(4):
                        nc.tensor.matmul(psk[:, mm, :], wt[:, jj, mm * P:(mm + 1) * P],
                                         kT[:, jj, :], start=(jj == 0), stop=(jj == 3))

                qf = work.tile([P, 4, 256], f16, tag="qf")
                kf = work.tile([P, 4, 256], f16, tag="kf")
                if "noact" not in ABL:
                    nc.scalar.copy(qf, psq)
                    nc.scalar.copy(kf, psk)

                # products: Pa = QA.KA + QB.KB ; Pb = QB.KA - QA.KB
                pt = work.tile([P, 4, 256], f16, tag="pt")
                if "noprod" not in ABL:
                    t1 = work.tile([P, 2, 256], f16, tag="t1")
                    t2 = work.tile([P, 2, 256], f16, tag="t2")
                    QA, QB = qf[:, 0:2, :], qf[:, 2:4, :]
                    KA, KB = kf[:, 0:2, :], kf[:, 2:4, :]
                    nc.vector.tensor_mul(t1, QA, KA)
                    nc.vector.tensor_mul(t2, QB, KB)
                    nc.vector.tensor_add(pt[:, 0:2, :], t1, t2)
                    nc.vector.tensor_mul(t1, QB, KA)
                    nc.vector.tensor_mul(t2, QA, KB)
                    nc.vector.tensor_sub(pt[:, 2:4, :], t1, t2)
                    # f=0 fixups (partition 0 of slices 0 and 2), one strided op
                    nc.vector.tensor_mul(
                        pt[0:1, 0:4:2, :], qf[0:1, 0:4:2, :], kf[0:1, 0:4:2, :])
                else:
                    nc.vector.tensor_copy(pt, qf)

                psc = psp.tile([P, 2, D], f32, tag="ps2bank")
                for sp in range(2):
                    cps = psc[:, sp, :]
                    for ff in range(4):
                        nc.tensor.matmul(cps, pt[:, ff, sp * P:(sp + 1) * P],
                                         ct[:, ff, :], start=(ff == 0), stop=(ff == 3))
                    mx = small.tile([P, 8], f32, tag="mx")
                    nc.vector.max(out=mx, in_=cps)
                    sm = small.tile([P, 1], f32, tag="sm")
                    nc.vector.reduce_sum(sm, mx[:, 0:TOPK], axis=mybir.AxisListType.X)
                    pm = small.tile([P, 1], f32, tag="pm")
                    nc.vector.tensor_scalar_mul(pm, sm, 1.0 / TOPK)
                    nc.scalar.activation(w1sb[:, 2 * gl + sp, :], cps,
                                         mybir.ActivationFunctionType.Sigmoid,
                                         bias=pm, scale=-1.0)

            def combine_super(v16, w1sb, vnext0, o16):
                """o16[:, s] = v16[:, s] + w1sb[:, s]*(v16[:, s+1] - v16[:, s]);
                s=7 uses vnext0."""
                for sl in range(SB_GROUP):
                    vnext = v16[:, sl + 1, :] if sl < SB_GROUP - 1 else vnext0
                    dt_ = work.tile([P, D], f16, tag="dt")
                    zt = work.tile([P, D], f16, tag="zt")
                    nc.vector.tensor_sub(dt_, vnext, v16[:, sl, :])
                    nc.vector.tensor_mul(zt, w1sb[:, sl, :], dt_)
                    nc.gpsimd.tensor_add(o16[:, sl, :], v16[:, sl, :], zt)

            def pipeline():
                prev = None  # (v16, o16, w1sb, sbi)
                for sbi in range(NSUPER):
                    q16, k16, v16 = load_super(sbi)
                    o16 = io.tile([P, SB_GROUP, D], f16, tag="o16")
                    w1sb = work.tile([P, SB_GROUP, D], f16, tag="w1sb", bufs=2)
                    for gl in range(4):
                        compute_group(q16, k16, gl, w1sb)
                    if prev is not None:
                        pv, po, pw, psbi = prev
                        combine_super(pv, pw, v16[:, 0, :], po)
                        nc.gpsimd.dma_start(
                            out=ov[:, psbi * SB_GROUP:(psbi + 1) * SB_GROUP, :], in_=po)
                    prev = (v16, o16, w1sb, sbi)

                pv, po, pw, psbi = prev
                combine_super(pv, pw, vsh, po)
                nc.gpsimd.dma_start(
                    out=ov[:, psbi * SB_GROUP:(psbi + 1) * SB_GROUP, :], in_=po)

            if n_iter == 1:
                pipeline()
            else:
                with tc.For_i(0, n_iter, 1):
                    pipeline()

    nc.finalize()
    return nc


def kernel(query, key, value):
    import sys
    if "/opt/trn_rl_repo" not in sys.path:
        sys.path.insert(0, "/opt/trn_rl_repo")
    from concourse.bass_utils import run_bass_kernel_spmd

    if "nc" not in _CACHE:
        _CACHE["nc"] = _build_nc()
    nc = _CACHE["nc"]

    q = np.ascontiguousarray(np.asarray(query, dtype=np.float32).reshape(B, L, D))
    k = np.ascontiguousarray(np.asarray(key, dtype=np.float32).reshape(B, L, D))
    v = np.ascontiguousarray(np.asarray(value, dtype=np.float32).reshape(B, L, D))

    in_maps = []
    for c in range(N_CORES):
        sl = slice(c * BPC, (c + 1) * BPC)
        in_maps.append({
            "query": q[sl].reshape(ROWS, D),
            "key": k[sl].reshape(ROWS, D),
            "value": v[sl].reshape(ROWS, D),
        })
    res = run_bass_kernel_spmd(nc, in_maps, core_ids=list(range(N_CORES)),
                               trace=bool(_CACHE.get("trace")))
    _CACHE["last_result"] = res
    out = np.empty((B, L, D), dtype=np.float32)
    for c in range(N_CORES):
        out[c * BPC:(c + 1) * BPC] = res.results[c]["out"].reshape(BPC, L, D)
    return out


# revision 22
# speedup vs baseline: 200.6383x; 1.0424x over previous
"""AutoCorrelation Trainium2 kernel (Bass/Tile, 8 NeuronCores).

Math (per row r of [B*L, 512] with D=512):
  corr_r = irfft(rfft(q_r) * conj(rfft(k_r)))            (circular cross-correlation)
  mean_r = mean(top7(corr_r))
  w0 = sigmoid(corr - mean); out = v*w0 + roll(v,-1,L)*(1-w0)
     = v + sigmoid(mean - corr) * (roll(v) - v)

Implementation:
  - DFT/iDFT as fp16 matmuls on the PE with a packed-real 512-point basis:
    packed[f] layout: A-block f=0..255 = Re[f] (A[0]=Re0), B-block = Im[f]
    (B[0]=Re256).  Forward: QF^T[fpacked, row] = W^T q^T via
    lhsT=W-block, rhs=qT (DMA-xbar-transposed q16).  Product spectrum
    P = QF o conj(KF) elementwise on DVE (block formulas + 2 f=0 fixups).
    Inverse: corr[row, t] via lhsT=P-chunk, rhs=C-block -> PSUM fp32,
    already in row-major layout.
  - top-7 mean via the DVE max8 instruction reading corr in PSUM.
  - sigmoid on ACT directly off PSUM with per-partition bias = +mean/scale=-1.
  - Row interleave: partition p = row//64, subblock s = row%64 makes
    roll(v,-1) = "read subblock s+1" (same partitions); batch wraps and the
    s=63 edge are handled by one small shifted DMA load (vsh).
  - Sharding: batch-parallel, 4 batches per core, no communication.
"""
import numpy as np

B, L, D = 32, 2048, 512
N_CORES = 8
BPC = B // N_CORES            # batches per core
ROWS = BPC * L                # 8192 rows per core
NSUB = 64                     # subblocks (s = row % 64)
P = 128                       # partitions (p = row // 64)
SB_GROUP = 8                  # subblocks per DMA superblock
NSUPER = NSUB // SB_GROUP     # 8 superblocks
TOPK = 7

_CACHE = {}


def _dft_consts():
    """Packed-real DFT matrices W [512 feat, 512 packed] and C [512 packed, 512 t]."""
    j = np.arange(D)[:, None].astype(np.float64)
    f = np.arange(256)[None, :].astype(np.float64)
    Wc = np.cos(-2 * np.pi * j * f / D)
    Ws = np.sin(-2 * np.pi * j * f / D)
    WB = Ws.copy()
    WB[:, 0] = np.cos(np.pi * j[:, 0])          # B0 row: Re256
    W = np.concatenate([Wc, WB], axis=1)        # [512, 512]
    t = np.arange(D)[None, :].astype(np.float64)
    fc = np.arange(256)[:, None].astype(np.float64)
    Ca = np.cos(2 * np.pi * fc * t / D) * 2 / D
    Ca[0] = 1.0 / D
    Cb = -np.sin(2 * np.pi * fc * t / D) * 2 / D
    Cb[0] = np.cos(np.pi * t[0]) / D
    C = np.concatenate([Ca, Cb], axis=0)        # [512, 512]
    return W.astype(np.float32), C.astype(np.float32)


def _build_nc(n_iter=1):
    import os
    import concourse.bacc as bacc
    import concourse.mybir as mybir
    from concourse.tile import TileContext

    ABL = set(os.environ.get("AUTOCORR_ABL", "").split(","))

    f16 = mybir.dt.float16
    f32 = mybir.dt.float32

    W, C = _dft_consts()
    # W16[p, jj, fp]  = W[jj*128+p, fp]   (lhsT blocks for GEMM-1)
    W16 = W.reshape(4, P, D).transpose(1, 0, 2).astype(np.float16).copy()
    # C16[p, ff, t]   = C[ff*128+p, t]    (rhs blocks for GEMM-2)
    C16 = C.reshape(4, P, D).transpose(1, 0, 2).astype(np.float16).copy()

    nc = bacc.Bacc()
    q_d = nc.dram_tensor("query", [ROWS, D], f32, kind="ExternalInput")
    k_d = nc.dram_tensor("key", [ROWS, D], f32, kind="ExternalInput")
    v_d = nc.dram_tensor("value", [ROWS, D], f32, kind="ExternalInput")
    o_d = nc.dram_tensor("out", [ROWS, D], f32, kind="ExternalOutput")
    w_t = nc.inline_tensor(W16, name="Wdft")
    c_t = nc.inline_tensor(C16, name="Cdft")

    # interleaved views: [p, s, c] with row = 64*p + s
    qv = q_d.rearrange("(p s) c -> p s c", s=NSUB)
    kv = k_d.rearrange("(p s) c -> p s c", s=NSUB)
    vv = v_d.rearrange("(p s) c -> p s c", s=NSUB)
    ov = o_d.rearrange("(p s) c -> p s c", s=NSUB)

    with TileContext(nc) as tc:
        with (
            tc.tile_pool(name="consts", bufs=1) as consts,
            tc.tile_pool(name="io", bufs=2) as io,
            tc.tile_pool(name="work", bufs=3) as work,
            tc.tile_pool(name="small", bufs=8) as small,
            tc.tile_pool(name="ps", bufs=3, space="PSUM") as psp,
            tc.tile_pool(name="pscb", bufs=2, space="PSUM") as pscp,
        ):
            wt = consts.tile([P, 4, D], f16)      # W16
            ct = consts.tile([P, 4, D], f16)      # C16
            nc.sync.dma_start(out=wt, in_=w_t[:, :, :])
            nc.sync.dma_start(out=ct, in_=c_t[:, :, :])

            # vsh[p] = v[row 64p+64] ; fix wraps at p in {31,63,95,127} <- batch starts
            vsh = consts.tile([P, D], f16)
            vflat = v_d  # [ROWS, D]
            nc.gpsimd.dma_start(
                out=vsh[0:127], in_=vflat.rearrange("(a b) c -> a b c", b=NSUB)[1:128, 0]
            )  # rows 64,128,...,8128
            nc.gpsimd.dma_start(
                out=vsh.rearrange("(w u) c -> w u c", u=32)[:, 31:32, :].rearrange("w u c -> (w u) c"),
                in_=vflat.rearrange("(b t) c -> b t c", t=L)[:, 0:1, :].rearrange("b t c -> (b t) c"),
            )  # vsh[31,63,95,127] <- v rows {0, 2048, 4096, 6144}

            def load_super(sbi):
                sl = slice(sbi * SB_GROUP, (sbi + 1) * SB_GROUP)
                q16 = io.tile([P, SB_GROUP, D], f16, tag="q16")
                k16 = io.tile([P, SB_GROUP, D], f16, tag="k16")
                v16 = io.tile([P, SB_GROUP, D], f16, tag="v16")
                if "loadhalf" in ABL:
                    nc.gpsimd.dma_start(out=q16, in_=qv[:, sl, :])
                    return q16, q16, q16
                nc.gpsimd.dma_start(out=q16, in_=qv[:, sl, :])
                nc.gpsimd.dma_start(out=k16, in_=kv[:, sl, :])
                nc.gpsimd.dma_start(out=v16, in_=vv[:, sl, :])
                return q16, k16, v16

            def compute_group(q16, k16, gl, w1sb):
                """gl: local group index (0..3) inside superblock; writes w1 into
                w1sb slices for the two subblocks (local indices 2gl, 2gl+1)."""
                qT = work.tile([P, 4, 256], f16, tag="qT")
                kT = work.tile([P, 4, 256], f16, tag="kT")
                for sp in range(2):
                    nc.sync.dma_start_transpose(
                        qT[:, :, sp * P:(sp + 1) * P], q16[:, 2 * gl + sp, :])
                    nc.sync.dma_start_transpose(
                        kT[:, :, sp * P:(sp + 1) * P], k16[:, 2 * gl + sp, :])

                psq = psp.tile([P, 4, 256], f32, tag="ps2bank")
                psk = psp.tile([P, 4, 256], f32, tag="ps2bank")
                for mm in range(4):
                    for jj in range(4):
                        nc.tensor.matmul(psq[:, mm, :], wt[:, jj, mm * P:(mm + 1) * P],
                                         qT[:, jj, :], start=(jj == 0), stop=(jj == 3))
                for mm in range(4):
                    for jj in range(4):
                        nc.tensor.matmul(psk[:, mm, :], wt[:, jj, mm * P:(mm + 1) * P],
                                         kT[:, jj, :], start=(jj == 0), stop=(jj == 3))

                qf = work.tile([P, 4, 256], f16, tag="qf")
                kf = work.tile([P, 4, 256], f16, tag="kf")
                if "noact" not in ABL:
                    nc.scalar.copy(qf, psq)
                    nc.scalar.copy(kf, psk)

                # products: Pa = QA.KA + QB.KB ; Pb = QB.KA - QA.KB
                pt = work.tile([P, 4, 256], f16, tag="pt")
                if "noprod" not in ABL:
                    t1 = work.tile([P, 2, 256], f16, tag="t1")
                    t2 = work.tile([P, 2, 256], f16, tag="t2")
                    QA, QB = qf[:, 0:2, :], qf[:, 2:4, :]
                    KA, KB = kf[:, 0:2, :], kf[:, 2:4, :]
                    nc.vector.tensor_mul(t1, QA, KA)
                    nc.vector.tensor_mul(t2, QB, KB)
                    nc.vector.tensor_add(pt[:, 0:2, :], t1, t2)
                    nc.vector.tensor_mul(t1, QB, KA)
                    nc.vector.tensor_mul(t2, QA, KB)
                    nc.vector.tensor_sub(pt[:, 2:4, :], t1, t2)
                    # f=0 fixups (partition 0 of slices 0 and 2), one strided op
                    nc.vector.tensor_mul(
                        pt[0:1, 0:4:2, :], qf[0:1, 0:4:2, :], kf[0:1, 0:4:2, :])
                else:
                    nc.vector.tensor_copy(pt, qf)

                for sp in range(2):
                    cps = pscp.tile([P, D], f32, tag="psc1bank")
                    for ff in range(4):
                        nc.tensor.matmul(cps, pt[:, ff, sp * P:(sp + 1) * P],
                                         ct[:, ff, :], start=(ff == 0), stop=(ff == 3))
                    mx = small.tile([P, 8], f32, tag="mx")
                    nc.vector.max(out=mx, in_=cps)
                    sm = small.tile([P, 1], f32, tag="sm")
                    nc.vector.reduce_sum(sm, mx[:, 0:TOPK], axis=mybir.AxisListType.X)
                    pm = small.tile([P, 1], f32, tag="pm")
                    nc.vector.tensor_scalar_mul(pm, sm, 1.0 / TOPK)
                    nc.scalar.activation(w1sb[:, 2 * gl + sp, :], cps,
                                         mybir.ActivationFunctionType.Sigmoid,
                                         bias=pm, scale=-1.0)

            def combine_super(v16, w1sb, vnext0, o16):
                """o16[:, s] = v16[:, s] + w1sb[:, s]*(v16[:, s+1] - v16[:, s]);
                s=7 uses vnext0."""
                for sl in range(SB_GROUP):
                    vnext = v16[:, sl + 1, :] if sl < SB_GROUP - 1 else vnext0
                    dt_ = work.tile([P, D], f16, tag="dt")
                    zt = work.tile([P, D], f16, tag="zt")
                    nc.vector.tensor_sub(dt_, vnext, v16[:, sl, :])
                    nc.vector.tensor_mul(zt, w1sb[:, sl, :], dt_)
                    nc.vector.tensor_add(o16[:, sl, :], v16[:, sl, :], zt)

            def pipeline():
                prev = None  # (v16, o16, w1sb, sbi)
                for sbi in range(NSUPER):
                    q16, k16, v16 = load_super(sbi)
                    o16 = io.tile([P, SB_GROUP, D], f16, tag="o16")
                    w1sb = work.tile([P, SB_GROUP, D], f16, tag="w1sb", bufs=2)
                    for gl in range(4):
                        compute_group(q16, k16, gl, w1sb)
                    if prev is not None:
                        pv, po, pw, psbi = prev
                        combine_super(pv, pw, v16[:, 0, :], po)
                        nc.gpsimd.dma_start(
                            out=ov[:, psbi * SB_GROUP:(psbi + 1) * SB_GROUP, :], in_=po)
                    prev = (v16, o16, w1sb, sbi)

                pv, po, pw, psbi = prev
                combine_super(pv, pw, vsh, po)
                nc.gpsimd.dma_start(
                    out=ov[:, psbi * SB_GROUP:(psbi + 1) * SB_GROUP, :], in_=po)

            if n_iter == 1:
                pipeline()
            else:
                with tc.For_i(0, n_iter, 1):
                    pipeline()

    nc.finalize()
    return nc


def kernel(query, key, value):
    import sys
    if "/opt/trn_rl_repo" not in sys.path:
        sys.path.insert(0, "/opt/trn_rl_repo")
    from concourse.bass_utils import run_bass_kernel_spmd

    if "nc" not in _CACHE:
        _CACHE["nc"] = _build_nc()
    nc = _CACHE["nc"]

    q = np.ascontiguousarray(np.asarray(query, dtype=np.float32).reshape(B, L, D))
    k = np.ascontiguousarray(np.asarray(key, dtype=np.float32).reshape(B, L, D))
    v = np.ascontiguousarray(np.asarray(value, dtype=np.float32).reshape(B, L, D))

    in_maps = []
    for c in range(N_CORES):
        sl = slice(c * BPC, (c + 1) * BPC)
        in_maps.append({
            "query": q[sl].reshape(ROWS, D),
            "key": k[sl].reshape(ROWS, D),
            "value": v[sl].reshape(ROWS, D),
        })
    res = run_bass_kernel_spmd(nc, in_maps, core_ids=list(range(N_CORES)),
                               trace=bool(_CACHE.get("trace")))
    _CACHE["last_result"] = res
    out = np.empty((B, L, D), dtype=np.float32)
    for c in range(N_CORES):
        out[c * BPC:(c + 1) * BPC] = res.results[c]["out"].reshape(BPC, L, D)
    return out
